# revision 44
# baseline (speedup 1.0000x reference)
"""GCN 2-layer decoder on 8 trn2 NeuronCores.

Algorithm (per core, nodes dest-sharded):
  deg[c]  = sum of in-edge weights (+1 self loop)   [host pads slots, DVE reduce]
  dinv    = 1/sqrt(deg)
  xt1[r]  = dinv[r] * (z @ W1)[r]      -> bf16 rows in a Shared DRAM table
  agg[c]  = xt1[c] + sum_e ew_e * xt1[row_e]
            [self-loop via identity-matmul PSUM init; edges via dma_gather
             rows + selector-matmul accumulated per half directly in PSUM]
  h1s[c]  = relu(dinv[c]*agg[c] + b1) * dinv[c]
  xt2[r]  = (h1s @ W2)[r]              -> bf16 rows in Shared table
  out[c]  = dinv[c] * (xt2[c] + sum_e ew_e * xt2[row_e]) + b2

Self-loop edges are excluded from the gathered edge list (their contribution
is the ident-matmul PSUM init).  Remaining edges are sorted by (dest-half,
source-quarter, dest-block); each (half, quarter, block) run is padded to a
uniform (cross-core) tile count so the single SPMD program works on all 8
cores.  Source rows are fetched with gpsimd.dma_gather round-robined over 4
SWDGE queues (each queue's descriptor generation runs on its own Q7 cpu
pair, so up to 4 gathers generate descriptors concurrently); per 128-edge
tile a [128,128] bf16 selector S (S[e,d] = ew_e * (d == dloc_e%128)) is
built 8 tiles at a time with two wide DVE tensor_tensor ops (stride-0
broadcast of the per-tile dloc/ew columns) and PE accumulates S.T @ G into
the block's PSUM column with start/stop spanning the whole half.
"""

import math
from contextlib import ExitStack
from dataclasses import dataclass

import numpy as np

P = 128
KB = 8  # tiles per batched selector build


@dataclass(frozen=True)
class Cfg:
    n: int              # total nodes
    ncores: int         # 8
    qn: int             # source quarters (index range per gather table slice)
    f_in: int           # 64
    f_hid: int          # 64
    f_out: int          # 32
    ch_tiles: int = 16  # gather chunk size in 128-edge tiles

    @property
    def nshard(self):
        return self.n // self.ncores

    @property
    def nblk(self):
        return math.ceil(self.nshard / P)

    @property
    def nblk_h(self):
        return math.ceil(self.nblk / 2)

    @property
    def dests_pad(self):
        return self.nblk * P

    @property
    def qsize(self):
        return self.n // self.qn


FULL_CFG = Cfg(n=100000, ncores=8, qn=4, f_in=64, f_hid=64, f_out=32)


# ---------------------------------------------------------------- host side

def preprocess(cfg: Cfg, edge_index: np.ndarray, edge_attr: np.ndarray):
    """Build the uniform schedule + per-core device input arrays."""
    import ml_dtypes

    n = cfg.n
    ns = cfg.nshard
    nbh = cfg.nblk_h

    # message edges exclude self loops (handled by ident-matmul PSUM init)
    rows = edge_index[0]
    cols = edge_index[1]
    ews = edge_attr.astype(np.float32)

    core = cols // ns
    dloc = (cols - core * ns).astype(np.int64)
    q = rows // cfg.qsize
    rloc = (rows - q * cfg.qsize).astype(np.int64)
    blk = dloc // P
    half = (blk >= nbh).astype(np.int64)
    bh = blk - half * nbh  # block within half

    assert rloc.max() < 32768, "quarter-local index must fit int16"

    # run id in schedule order: (half, quarter, block-in-half)
    run_id = (half * cfg.qn + q) * nbh + bh
    n_runs = 2 * cfg.qn * nbh

    # counts per (core, run)
    cnt = np.zeros((cfg.ncores, n_runs), dtype=np.int64)
    np.add.at(cnt, (core, run_id), 1)
    T = np.maximum(1, np.ceil(cnt.max(axis=0) / P).astype(np.int64))  # [n_runs]

    run_tile_off = np.concatenate([[0], np.cumsum(T)])   # tile offset per run
    total_tiles = int(run_tile_off[-1])                   # tiles per layer
    total_slots = total_tiles * P

    # per-run tile metadata (uniform across cores)
    tile_run = np.repeat(np.arange(n_runs), T)            # [total_tiles]
    t_q = (tile_run // nbh) % cfg.qn
    t_bh = tile_run % nbh
    t_j = np.arange(total_tiles) - run_tile_off[tile_run]
    last_j = T[tile_run] - 1
    t_stop = (t_q == cfg.qn - 1) & (t_j == last_j)

    # Tile schedule: within each (half, quarter) segment, sort tiles by
    # (j, run) so pad-lightest come first, then DEAL them round-robin over
    # the segment's gather chunks (each chunk then carries a similar share
    # of padding as a trailing suffix, marked idx=-1 below so the gather
    # ucode skips its descriptors).  Chunks of the 4 quarters are then
    # INTERLEAVED round-robin per half, with SWDGE queue == quarter, so the
    # 4 queues stream their quarters' chunks continuously without segment
    # boundaries.
    per_seg_chunks = {}
    for h in range(2):
        for qq in range(cfg.qn):
            r0 = (h * cfg.qn + qq) * nbh
            lo, hi = run_tile_off[r0], run_tile_off[r0 + nbh]
            seg = np.arange(lo, hi)
            s = seg[np.lexsort((tile_run[seg], t_j[seg]))]  # j asc, run asc
            st = len(seg)
            nch = math.ceil(st / cfg.ch_tiles)
            caps = [cfg.ch_tiles] * (nch - 1) + [st - (nch - 1) * cfg.ch_tiles]
            buckets = [[] for _ in range(nch)]
            bi = 0
            for t_ in s:
                while len(buckets[bi % nch]) >= caps[bi % nch]:
                    bi += 1
                buckets[bi % nch].append(t_)
                bi += 1
            per_seg_chunks[(h, qq)] = buckets

    order = []          # old tile index in global emission order
    chunk_meta = []     # (h, q, c0_within_segment, cht) per emitted chunk
    spos = np.empty(total_tiles, dtype=np.int64)  # segment-local tile pos
    for h in range(2):
        queues = [list(per_seg_chunks[(h, qq)]) for qq in range(cfg.qn)]
        cursors = [0] * cfg.qn
        while any(queues):
            for qq in range(cfg.qn):
                if not queues[qq]:
                    continue
                b = queues[qq].pop(0)
                chunk_meta.append((h, qq, cursors[qq], len(b)))
                for i, t_ in enumerate(b):
                    spos[t_] = cursors[qq] + i
                cursors[qq] += len(b)
                order.extend(b)
    order = np.asarray(order, dtype=np.int64)
    gpos = np.empty(total_tiles, dtype=np.int64)
    gpos[order] = np.arange(total_tiles)
    tile_run = tile_run[order]
    t_bh = t_bh[order]
    t_half = tile_run // (cfg.qn * nbh)
    # stop flag: each block's LAST PSUM write in the emission order
    t_stop = np.zeros(total_tiles, dtype=bool)
    last_of_block = {}
    for p in range(total_tiles):
        last_of_block[(int(t_half[p]), int(t_bh[p]))] = p
    for p in last_of_block.values():
        t_stop[p] = True

    # per-(half,q) segment sizes in tile units
    seg_tiles = {}
    for h in range(2):
        for qq in range(cfg.qn):
            r0 = (h * cfg.qn + qq) * nbh
            seg_tiles[(h, qq)] = int(T[r0:r0 + nbh].sum())

    sched = {
        "T": T, "tile_run": tile_run, "t_bh": t_bh, "t_stop": t_stop,
        "run_tile_off": run_tile_off, "total_tiles": total_tiles,
        "seg_tiles": seg_tiles, "chunk_meta": chunk_meta,
    }

    # degree includes self loops (weight 1)
    cols_deg = np.concatenate([cols, np.arange(n, dtype=np.int64)])
    ews_deg = np.concatenate([ews, np.ones(n, dtype=np.float32)])
    deg_cnt = np.bincount(cols_deg, minlength=n)
    dslot = int(math.ceil((deg_cnt.max() + 1) / 8) * 8)
    sched["dslot"] = dslot

    per_core = []
    order_all = np.lexsort((dloc, run_id, core))  # sorted by core, run, dloc
    core_sorted = core[order_all]
    core_bounds = np.searchsorted(core_sorted, np.arange(cfg.ncores + 1))

    for c in range(cfg.ncores):
        sel = order_all[core_bounds[c]:core_bounds[c + 1]]
        c_run = run_id[sel]
        c_rloc = rloc[sel]
        c_dloc = dloc[sel]
        c_ew = ews[sel]

        # rank within run (sel is sorted by run)
        run_starts = np.searchsorted(c_run, np.arange(n_runs))
        rank = np.arange(len(sel)) - run_starts[c_run]
        tile_old = run_tile_off[c_run] + rank // P
        slot = (gpos[tile_old] * P + rank % P).astype(np.int64)   # global
        sslot = (spos[tile_old] * P + rank % P).astype(np.int64)  # in-segment
        e_seg = c_run // nbh                                      # h*qn + q

        s_dlocrel = np.zeros(total_slots, dtype=np.float32)
        s_ew = np.zeros(total_slots, dtype=np.float32)
        occupied = np.zeros(total_slots, dtype=bool)
        s_dlocrel[slot] = (c_dloc % P).astype(np.float32)
        s_ew[slot] = c_ew
        occupied[slot] = True

        rloc_seg = {}
        for h in range(2):
            for qq in range(cfg.qn):
                rloc_seg[(h, qq)] = np.zeros(seg_tiles[(h, qq)] * P,
                                             dtype=np.int16)
        for h in range(2):
            for qq in range(cfg.qn):
                m = e_seg == h * cfg.qn + qq
                rloc_seg[(h, qq)][sslot[m]] = c_rloc[m].astype(np.int16)

        # Mark each gather chunk's trailing unoccupied slots idx=-1 and
        # record the per-chunk occupied-prefix length.  The gather ucode
        # trims trailing negatives on the generation side; the count must
        # also reach the decode side via num_idxs_reg so the ring
        # reservation matches what generation actually pushes.
        ccnt = []
        g0 = 0
        for (h, qq, c0, cht) in chunk_meta:
            occ = occupied[g0 * P:(g0 + cht) * P]
            nz = np.nonzero(occ)[0]
            tail = 0 if len(nz) == 0 else int(nz[-1]) + 1
            rloc_seg[(h, qq)][c0 * P + tail:(c0 + cht) * P] = -1
            ccnt.append(tail)
            g0 += cht
        ccnt = np.asarray(ccnt, dtype=np.int32)[None, :]

        # dense selector table: S[e, t*128+d] = ew_slot * (dloc_slot == d),
        # slot = t*128+e.  Loaded straight from DRAM instead of being built
        # on DVE per tile (same table serves both layers).
        sd = s_dlocrel.reshape(total_tiles, P).astype(np.int64)
        se = s_ew.reshape(total_tiles, P).astype(np.float32)
        S = np.zeros((total_tiles, P, P), dtype=np.float32)
        S[np.arange(total_tiles)[:, None], np.arange(P)[None, :], sd] = se
        s_tab = np.ascontiguousarray(
            S.transpose(1, 0, 2).reshape(P, total_tiles * P)).astype(
                ml_dtypes.bfloat16)
        del S

        # idx arrays per (h,q) segment, wrapped 16 + replicated to 128 parts
        idx_segs = {}
        for h in range(2):
            for qq in range(cfg.qn):
                seg = rloc_seg[(h, qq)]
                wrapped = np.ascontiguousarray(seg.reshape(-1, 16).T)
                idx_segs[(h, qq)] = np.ascontiguousarray(
                    np.tile(wrapped, (P // 16, 1)))

        # degree pad array [128, nblk*dslot]
        dmask = (cols_deg // ns) == c
        dd = (cols_deg[dmask] - c * ns).astype(np.int64)
        dw = ews_deg[dmask]
        o2 = np.argsort(dd, kind="stable")
        dd, dw = dd[o2], dw[o2]
        dstart = np.searchsorted(dd, np.arange(ns))
        drank = np.arange(len(dd)) - dstart[dd]
        degpad = np.zeros((P, cfg.nblk * dslot), dtype=np.float32)
        degpad[dd % P, (dd // P) * dslot + drank] = dw
        # phantom dests get deg=1 to avoid 1/0
        for ph in range(ns, cfg.dests_pad):
            degpad[ph % P, (ph // P) * dslot] = 1.0

        per_core.append({
            "s_tab": s_tab, "idx_segs": idx_segs,
            "degpad": degpad, "ccnt": ccnt,
        })

    return sched, per_core


# ---------------------------------------------------------------- device side

def build_program(cfg: Cfg, sched, dbg: bool = False):
    import ml_dtypes  # noqa: F401
    from concourse import bacc, bass, mybir, tile
    from concourse.library_config import mlp

    f32 = mybir.dt.float32
    bf16 = mybir.dt.bfloat16
    i16 = mybir.dt.int16
    Alu = mybir.AluOpType
    Act = mybir.ActivationFunctionType

    n, ns, nbh, nblk = cfg.n, cfg.nshard, cfg.nblk_h, cfg.nblk
    dslot = sched["dslot"]
    TT = sched["total_tiles"]
    f_in, f_hid, f_out = cfg.f_in, cfg.f_hid, cfg.f_out

    nc = bacc.Bacc("TRN2", target_bir_lowering=False, debug=False,
                   enable_asserts=False, num_devices=cfg.ncores,
                   num_swdge_queues=4)

    # ---- I/O declarations
    zT_d = nc.dram_tensor("zT", [f_in, ns], f32, kind="ExternalInput")
    w1_d = nc.dram_tensor("W1", [f_in, f_hid], f32, kind="ExternalInput")
    w2_d = nc.dram_tensor("W2", [f_hid, f_out], f32, kind="ExternalInput")
    b1b_d = nc.dram_tensor("b1b", [P, f_hid], f32, kind="ExternalInput")
    b2b_d = nc.dram_tensor("b2b", [P, f_out], f32, kind="ExternalInput")
    ident_d = nc.dram_tensor("ident", [P, P], f32, kind="ExternalInput")
    identb_d = nc.dram_tensor("identb", [P, P], bf16, kind="ExternalInput")
    degpad_d = nc.dram_tensor("degpad", [P, nblk * dslot], f32,
                              kind="ExternalInput")
    stab_d = nc.dram_tensor("stab", [P, TT * P], bf16, kind="ExternalInput")
    n_chunks = sum(math.ceil(sched["seg_tiles"][(h, qq)] / cfg.ch_tiles)
                   for h in range(2) for qq in range(cfg.qn))
    ccnt_d = nc.dram_tensor("ccnt", [1, n_chunks], mybir.dt.int32,
                            kind="ExternalInput")
    idx_d = {}
    for h in range(2):
        for qq in range(cfg.qn):
            st = sched["seg_tiles"][(h, qq)]
            idx_d[(h, qq)] = nc.dram_tensor(
                f"idx_h{h}q{qq}", [P, st * P // 16], i16, kind="ExternalInput")
    out_d = nc.dram_tensor("out", [cfg.dests_pad, f_out], f32,
                           kind="ExternalOutput")
    if dbg:
        dbg_xg1 = nc.dram_tensor("dbg_xg1", [n, P], bf16,
                                 kind="ExternalOutput")
        dbg_h1s = nc.dram_tensor("dbg_h1s", [P, nblk * f_hid], f32,
                                 kind="ExternalOutput")
        dbg_dinv = nc.dram_tensor("dbg_dinv", [P, nblk], f32,
                                  kind="ExternalOutput")

    # local slice + shared gathered tables (rows padded to 128 bf16 = 256B)
    xloc1 = nc.dram_tensor("xloc1", [ns, P], bf16, kind="Internal")
    xg1 = nc.dram_tensor("xg1", [n, P], bf16, kind="Internal",
                         addr_space="Shared")
    xloc2 = nc.dram_tensor("xloc2", [ns, P], bf16, kind="Internal")
    xg2 = nc.dram_tensor("xg2", [n, P], bf16, kind="Internal",
                         addr_space="Shared")

    groups = [list(range(cfg.ncores))]

    with tile.TileContext(nc, num_cores=cfg.ncores) as tc, \
            ExitStack() as ctx:
        nc.gpsimd.load_library(mlp)

        cpool = ctx.enter_context(tc.tile_pool(name="const", bufs=1))

        def load_const(dram, shape, dtype, tag):
            t = cpool.tile(shape, dtype, tag=tag)
            nc.sync.dma_start(out=t[:], in_=dram[:])
            return t

        ident_sb = load_const(ident_d, [P, P], f32, "ident")
        identb_sb = load_const(identb_d, [P, P], bf16, "identb")
        ccnt_sb = load_const(ccnt_d, [1, n_chunks], mybir.dt.int32, "ccnt")
        b1b_sb = load_const(b1b_d, [P, f_hid], f32, "b1b")
        b2b_sb = load_const(b2b_d, [P, f_out], f32, "b2b")
        w1_sb = load_const(w1_d, [f_in, f_hid], f32, "w1")
        w2_sb = load_const(w2_d, [f_hid, f_out], f32, "w2")
        idx_sb = {}
        for h in range(2):
            for qq in range(cfg.qn):
                st = sched["seg_tiles"][(h, qq)]
                idx_sb[(h, qq)] = load_const(idx_d[(h, qq)],
                                             [P, st * P // 16], i16,
                                             f"idx{h}{qq}")

        # persistent bf16 copies of the shard's table rows (self-loop adds)
        xt1_sb = cpool.tile([P, nblk * f_hid], bf16, tag="xt1")
        xt2_sb = cpool.tile([P, nblk * f_out], bf16, tag="xt2")
        # last block has phantom rows the prep never writes; zero them
        nc.vector.memset(xt1_sb[:, (nblk - 1) * f_hid:nblk * f_hid], 0.0)
        nc.vector.memset(xt2_sb[:, (nblk - 1) * f_out:nblk * f_out], 0.0)

        # ---- deg -> dinv
        dinv_sb = cpool.tile([P, nblk], f32, tag="dinv")
        with tc.tile_pool(name="deg", bufs=1) as dpool:
            degpad_sb = dpool.tile([P, nblk * dslot], f32)
            nc.sync.dma_start(out=degpad_sb[:], in_=degpad_d[:])
            deg_sb = dpool.tile([P, nblk], f32)
            nc.vector.tensor_reduce(
                out=deg_sb[:],
                in_=degpad_sb[:].rearrange("p (b s) -> p b s", s=dslot),
                axis=mybir.AxisListType.X, op=Alu.add)
            rdeg_sb = dpool.tile([P, nblk], f32)
            nc.vector.reciprocal(out=rdeg_sb[:], in_=deg_sb[:])
            nc.scalar.activation(out=dinv_sb[:], in_=rdeg_sb[:], func=Act.Sqrt)

        # ---- xt1 = dinv * (z @ W1): write bf16 rows into xt1_sb + xloc1
        def emit_xt_prep(src_get, w_sb, fdim_in, fdim_out, xt_sb, xloc, scale):
            """src_get(chunk)->AP [fdim_in, width] feature-major source."""
            with tc.tile_pool(name="xprep", bufs=3) as xp, \
                    tc.tile_pool(name="xprep_ps", bufs=3, space="PSUM") as xps:
                nchunks = math.ceil(ns / 512)
                for ch in range(nchunks):
                    n0 = ch * 512
                    width = min(512, ns - n0)
                    ps_x = xps.tile([fdim_out, 512], f32, tag="ps_x")
                    nc.tensor.matmul(out=ps_x[:, :width], lhsT=w_sb[:],
                                     rhs=src_get(ch, width), start=True,
                                     stop=True)
                    xT = xp.tile([fdim_out, 512], f32, tag="xT")
                    nc.vector.tensor_copy(out=xT[:, :width], in_=ps_x[:, :width])
                    for j in range(math.ceil(width / P)):
                        nb = ch * 4 + j
                        w = min(P, width - j * P)
                        ps_t = xps.tile([P, fdim_out], f32, tag="ps_t")
                        nc.tensor.transpose(
                            out=ps_t[:w, :], in_=xT[:, j * P:j * P + w],
                            identity=ident_sb[:fdim_out, :fdim_out])
                        dst = xt_sb[:w, nb * fdim_out:(nb + 1) * fdim_out]
                        if scale:
                            nc.vector.tensor_scalar(
                                out=dst, in0=ps_t[:w, :],
                                scalar1=dinv_sb[:w, nb:nb + 1], scalar2=None,
                                op0=Alu.mult)
                        else:
                            nc.vector.tensor_copy(out=dst, in_=ps_t[:w, :])
                        nc.sync.dma_start(
                            out=xloc[n0 + j * P:n0 + j * P + w, 0:fdim_out],
                            in_=xt_sb[:w, nb * fdim_out:(nb + 1) * fdim_out])

        with tc.tile_pool(name="zt", bufs=1) as zpool:
            zT_sb = zpool.tile([f_in, ns], f32)
            nc.sync.dma_start(out=zT_sb[:], in_=zT_d[:])
            emit_xt_prep(lambda ch, w: zT_sb[:, ch * 512:ch * 512 + w],
                         w1_sb, f_in, f_hid, xt1_sb, xloc1, scale=True)

        nc.gpsimd.collective_compute(
            "AllGather", Alu.bypass, replica_groups=groups,
            ins=[xloc1[:]], outs=[xg1[:]])
        if dbg:
            nc.sync.dma_start(out=dbg_xg1[:], in_=xg1[:])
            nc.sync.dma_start(out=dbg_dinv[:], in_=dinv_sb[:])

        # ---- aggregation layer
        ctx_pools = {
            "g": ctx.enter_context(tc.tile_pool(name="gpool", bufs=17)),
            "s": ctx.enter_context(tc.tile_pool(name="spool", bufs=6)),
        }
        chunk_no = [0]
        # zero the gather buffers once: slots trimmed by the idx=-1 suffix
        # keep stale SBUF, and a NaN bit pattern would poison S=0 matmuls
        for _ in range(17):
            gz = ctx_pools["g"].tile([P, cfg.ch_tiles, P], bf16, tag="G")
            nc.vector.memset(gz[:], 0.0)
        # rotating registers for the per-chunk trimmed index count (the Q7
        # reads the scalar register at deferred decode, so rotate 8 deep)
        cnt_regs = [nc.gpsimd.alloc_register(f"gcnt{i}") for i in range(8)]

        def emit_agg(xg, fdim, xt_sb, epilogue, pspool):
            gpool = ctx_pools["g"]
            spool = ctx_pools["s"]
            g_emit = 0
            lc = 0
            cur_h = None
            ps_h = None

            def open_half(h):
                ps = pspool.tile([P, nbh * fdim], f32, tag="ps_h")
                # self-loop contribution initializes PSUM: ps[b] = xt[b].
                # One start=True matmul per 2KB PSUM bank (512 f32): a
                # second start=True in the same bank before a stop discards
                # the first session's contents.
                half_cols = nbh * fdim
                for col0 in range(0, half_cols, 512):
                    wcols = min(512, half_cols - col0)
                    nc.tensor.matmul(
                        out=ps[:, col0:col0 + wcols],
                        lhsT=identb_sb[:],
                        rhs=xt_sb[:, h * half_cols + col0:
                                  h * half_cols + col0 + wcols],
                        start=True, stop=False)
                return ps

            for (h, qq, c0, cht) in sched["chunk_meta"]:
                if h != cur_h:
                    if cur_h is not None:
                        epilogue(cur_h, ps_h)
                    cur_h = h
                    ps_h = open_half(h)
                ixs = idx_sb[(h, qq)]
                gt = gpool.tile([P, cfg.ch_tiles, P], bf16, tag="G")
                nidx = cht * P
                creg = cnt_regs[lc % 8]
                nc.gpsimd.reg_load(creg, ccnt_sb[0:1, lc:lc + 1])
                nc.gpsimd.dma_gather(
                    out_ap=gt[:, 0:cht, :],
                    in_ap=xg[qq * cfg.qsize:(qq + 1) * cfg.qsize, :],
                    idxs_ap=ixs[:, c0 * 8:(c0 + cht) * 8],
                    num_idxs=nidx, num_idxs_reg=creg, elem_size=P,
                    single_packet=False,
                    queue_num=qq)
                s_t = spool.tile([P, cfg.ch_tiles * P], bf16, tag="S")
                nc.scalar.dma_start(
                    out=s_t[:, 0:cht * P],
                    in_=stab_d[:, g_emit * P:(g_emit + cht) * P])
                for t in range(cht):
                    g = g_emit + t
                    b = int(sched["t_bh"][g])
                    nc.tensor.matmul(
                        out=ps_h[:, b * fdim:(b + 1) * fdim],
                        lhsT=s_t[:, t * P:(t + 1) * P],
                        rhs=gt[:, t, 0:fdim],
                        start=False,
                        stop=bool(sched["t_stop"][g]))
                g_emit += cht
                lc += 1
            epilogue(cur_h, ps_h)

        # L1 epilogue: h1s = relu(dinv*ps + b1) * dinv — four half-wide DVE
        # ops using stride-0 broadcasts of dinv (per block) and b1 (tiled)
        h1pool = ctx.enter_context(tc.tile_pool(name="h1s", bufs=1))
        h1s_sb = h1pool.tile([P, nblk * f_hid], f32)
        epool = ctx.enter_context(tc.tile_pool(name="epiw", bufs=1))

        def bcast_dinv(h, fdim):
            ap = dinv_sb[:, h * nbh:(h + 1) * nbh]  # [P, nbh]
            return bass.AP(ap.tensor, ap.offset,
                           [list(ap.ap[0]), list(ap.ap[1]), [0, fdim]])

        def tile_bias(b_sb, fdim):
            ap = b_sb[:]  # [P, fdim] -> broadcast [P, nbh, fdim]
            return bass.AP(ap.tensor, ap.offset,
                           [list(ap.ap[0]), [0, nbh], list(ap.ap[1])])

        def epi1(h, ps_h):
            hc = nbh * f_hid
            u = epool.tile([P, hc], f32, tag="u1")
            uv = u[:].rearrange("p (b f) -> p b f", b=nbh)
            psv = ps_h[:].rearrange("p (b f) -> p b f", b=nbh)
            hv = h1s_sb[:, h * hc:(h + 1) * hc].rearrange(
                "p (b f) -> p b f", b=nbh)
            nc.vector.tensor_tensor(out=uv, in0=psv,
                                    in1=bcast_dinv(h, f_hid), op=Alu.mult)
            nc.vector.tensor_tensor(out=uv, in0=uv,
                                    in1=tile_bias(b1b_sb, f_hid), op=Alu.add)
            nc.vector.tensor_scalar(out=uv, in0=uv, scalar1=0.0,
                                    scalar2=None, op0=Alu.max)
            nc.vector.tensor_tensor(out=hv, in0=uv,
                                    in1=bcast_dinv(h, f_hid), op=Alu.mult)

        with tc.tile_pool(name="aggps1", bufs=1, space="PSUM") as pspool1:
            emit_agg(xg1, f_hid, xt1_sb, epi1, pspool1)
        if dbg:
            nc.sync.dma_start(out=dbg_h1s[:], in_=h1s_sb[:])

        # ---- xt2 = h1s @ W2 (h1s already carries the dinv source scale);
        # processed 4 blocks (512 nodes) per matmul to shorten the chain
        with tc.tile_pool(name="x2prep", bufs=3) as xp2, \
                tc.tile_pool(name="x2ps", bufs=2, space="PSUM") as xps2:
            for c4 in range(0, nblk, 4):
                nb4 = min(4, nblk - c4)
                ps_hT = xps2.tile([f_hid, 512], f32, tag="ps_hT")
                for j in range(nb4):
                    gb = c4 + j
                    w = min(P, ns - gb * P)
                    nc.tensor.transpose(
                        out=ps_hT[:, j * P:j * P + w],
                        in_=h1s_sb[:w, gb * f_hid:(gb + 1) * f_hid],
                        identity=ident_sb[:w, :w])
                wt = (nb4 - 1) * P + min(P, ns - (c4 + nb4 - 1) * P)
                hT = xp2.tile([f_hid, 512], f32, tag="hT")
                nc.vector.tensor_copy(out=hT[:, :wt], in_=ps_hT[:, :wt])
                ps_x2 = xps2.tile([f_out, 512], f32, tag="ps_x2")
                nc.tensor.matmul(out=ps_x2[:, :wt], lhsT=w2_sb[:],
                                 rhs=hT[:, :wt], start=True, stop=True)
                x2T = xp2.tile([f_out, 512], f32, tag="x2T")
                nc.vector.tensor_copy(out=x2T[:, :wt], in_=ps_x2[:, :wt])
                for j in range(nb4):
                    gb = c4 + j
                    w = min(P, ns - gb * P)
                    ps_t2 = xps2.tile([P, f_out], f32, tag="ps_t2")
                    nc.tensor.transpose(out=ps_t2[:w, :],
                                        in_=x2T[:, j * P:j * P + w],
                                        identity=ident_sb[:f_out, :f_out])
                    nc.vector.tensor_copy(
                        out=xt2_sb[:w, gb * f_out:(gb + 1) * f_out],
                        in_=ps_t2[:w, :])
                    nc.sync.dma_start(
                        out=xloc2[gb * P:gb * P + w, 0:f_out],
                        in_=xt2_sb[:w, gb * f_out:(gb + 1) * f_out])

        nc.gpsimd.collective_compute(
            "AllGather", Alu.bypass, replica_groups=groups,
            ins=[xloc2[:]], outs=[xg2[:]])

        # L2 epilogue: out = dinv*ps + b2 -> DRAM (two half-wide DVE ops,
        # then one DMA per block)
        def epi2(h, ps_h):
            hc = nbh * f_out
            u = epool.tile([P, hc], f32, tag="u2")
            uv = u[:].rearrange("p (b f) -> p b f", b=nbh)
            psv = ps_h[:].rearrange("p (b f) -> p b f", b=nbh)
            nc.vector.tensor_tensor(out=uv, in0=psv,
                                    in1=bcast_dinv(h, f_out), op=Alu.mult)
            nc.vector.tensor_tensor(out=uv, in0=uv,
                                    in1=tile_bias(b2b_sb, f_out), op=Alu.add)
            for b in range(nbh):
                gb = h * nbh + b
                nc.sync.dma_start(out=out_d[gb * P:(gb + 1) * P, :],
                                  in_=u[:, b * f_out:(b + 1) * f_out])

        with tc.tile_pool(name="aggps2", bufs=2, space="PSUM") as pspool2:
            emit_agg(xg2, f_out, xt2_sb, epi2, pspool2)

    nc.compile()
    return nc


# ---------------------------------------------------------------- entry point

def _run(cfg: Cfg, z, edge_index, edge_attr, W1, b1, W2, b2, dbg=False):
    import ml_dtypes
    from concourse.bass_utils import run_bass_kernel_spmd

    import time as _time
    _t = _time.time()
    sched, per_core = preprocess(cfg, np.asarray(edge_index),
                                 np.asarray(edge_attr, dtype=np.float32))
    print(f"[kernel] preprocess {_time.time()-_t:.1f}s "
          f"tiles/layer={sched['total_tiles']}", flush=True)
    _t = _time.time()
    nc = build_program(cfg, sched, dbg=dbg)
    print(f"[kernel] build+schedule {_time.time()-_t:.1f}s", flush=True)

    z = np.asarray(z, dtype=np.float32)
    W1 = np.asarray(W1, dtype=np.float32)
    b1 = np.asarray(b1, dtype=np.float32)
    W2 = np.asarray(W2, dtype=np.float32)
    b2 = np.asarray(b2, dtype=np.float32)

    ident = np.eye(P, dtype=np.float32)
    identb = np.eye(P, dtype=np.float32).astype(ml_dtypes.bfloat16)
    b1b = np.tile(b1[None, :], (P, 1)).astype(np.float32)
    b2b = np.tile(b2[None, :], (P, 1)).astype(np.float32)

    in_maps = []
    for c in range(cfg.ncores):
        pc = per_core[c]
        zt = np.ascontiguousarray(
            z[c * cfg.nshard:(c + 1) * cfg.nshard, :].T)
        m = {
            "zT": zt, "W1": W1, "W2": W2, "b1b": b1b, "b2b": b2b,
            "ident": ident, "identb": identb,
            "degpad": pc["degpad"],
            "stab": pc["s_tab"], "ccnt": pc["ccnt"],
        }
        for h in range(2):
            for qq in range(cfg.qn):
                m[f"idx_h{h}q{qq}"] = pc["idx_segs"][(h, qq)]
        in_maps.append(m)

    _t = _time.time()
    res = run_bass_kernel_spmd(
        nc, in_maps, core_ids=list(range(cfg.ncores)),
        trace=bool(int(__import__("os").environ.get("KERNEL_TRACE", "0"))))
    print(f"[kernel] compile+run {_time.time()-_t:.1f}s", flush=True)

    out = np.concatenate(
        [res.results[c]["out"][:cfg.nshard] for c in range(cfg.ncores)], axis=0)
    return out.astype(np.float32), res


def kernel(z, edge_index, edge_attr, W1, b1, W2, b2):
    out, _ = _run(FULL_CFG, z, edge_index, edge_attr, W1, b1, W2, b2)
    return out
